# revision 64
# baseline (speedup 1.0000x reference)
"""Trainium2 Bass kernel for DeepGCNLayer(GENConv softmax-aggr) + encoder.

Computation (see reference):
  h  = relu(batchnorm(x))                       # BN0 over all N nodes
  msg_e = relu(h[src_e]) + eps = h[src_e] + eps # h >= 0 already
  agg_v = softmax-weighted mean of msg over incoming edges (t=1)
  z0 = agg + h
  z1 = relu(BN1(z0 @ W1 + b1)); z2 = relu(BN2(z1 @ W2 + b2))
  out = (x + z2 @ W3 + b3) @ We + be

Strategy (8 NeuronCores, SPMD single program):
  * Host packs nodes into 128-slot tiles balanced by in-degree; edges are
    assigned to the core owning their dst node and packed into per-tile
    128-edge blocks.  The host PRE-GATHERS x[src] rows into edge-major
    bf16 tiles (pure indexing) and prebuilds the one-hot scatter matrix
    S (edge-slot -> dst-slot), so the device never does a per-edge
    gather (the previous dma_gather dominated at ~745us of GpSimd time)
    and needs no AllGather at all.
  * Device: BN0 stats via fused DVE reduce ops + tiny AllReduce; the
    per-channel scale/shift are broadcast across partitions with a
    transpose + K=1 ones-matmul so the per-edge affine can run in
    edge-major layout (DVE/GPSIMD tensor_tensor with free-dim broadcast).
  * Per node tile: y=xg*s (DVE), y2=y+t (GPSIMD), hb=relu (DVE),
    V=[exp(hb) | hb*exp(hb)] (ACT+DVE), then 12 chained PE matmuls
    S_b^T @ V_b accumulate [den|num] per dst slot; DVE recip+mult and a
    PE transpose produce agg, added to h for z0.
  * MLP: bf16 weight-stationary PE matmuls over 512-col blocks; BN sums
    via ACT accum_out, sum-of-squares via fused tensor_tensor_reduce;
    empty-slot corrected global BN via two more tiny AllReduces.
"""

import math
import numpy as np
from contextlib import ExitStack

import concourse.bass as bass
import concourse.tile as tile
from concourse import bacc, mybir, library_config
from concourse.bass_utils import run_bass_kernel_spmd
from concourse.masks import make_identity

F32 = mybir.dt.float32
BF16 = mybir.dt.bfloat16
AF = mybir.ActivationFunctionType
OP = mybir.AluOpType

NCORES = 8
EPS_BN = 1e-5
# den >= 1 for any slot with an incoming edge (each edge contributes
# exp(hb) >= 1), so this guard only fires on empty slots; keep it above the
# scalar-engine reciprocal's 2^-42 domain floor.
DEN_EPS = 1e-6

# full-size problem config (hardcoded; harness calls kernel() directly)
N_FULL = 50000
C = 128
H = 256


# --------------------------------------------------------------------------
# host-side graph partitioning (indexing only, no arithmetic on values)
# --------------------------------------------------------------------------

def _plan(n_nodes, src, dst):
    """Pack nodes into 128-slot bins balanced by in-degree; assign bins to
    cores; per (core, tile) edge blocks with slot assignments."""
    import heapq

    tiles = math.ceil(n_nodes / (NCORES * 128) + 0.02)  # slack for reserve
    if tiles * NCORES * 128 - NCORES < n_nodes:
        tiles += 1
    nbins = NCORES * tiles
    nloc = tiles * 128

    deg = np.bincount(dst, minlength=n_nodes).astype(np.int64)
    order = np.argsort(-deg, kind="stable")
    heap = [(0, i) for i in range(nbins)]
    heapq.heapify(heap)
    bin_nodes = [[] for _ in range(nbins)]
    bin_load = np.zeros(nbins, np.int64)
    for g in order:
        d = int(deg[g])
        spill = []
        while True:
            load, b = heapq.heappop(heap)
            if len(bin_nodes[b]) < 128:
                bin_nodes[b].append(g)
                bin_load[b] = load + d
                heapq.heappush(heap, (load + d, b))
                break
            spill.append((load, b))
        for it in spill:
            heapq.heappush(heap, it)

    # snake-assign bins to cores by load
    border = np.argsort(-bin_load, kind="stable")
    core_bins = [[] for _ in range(NCORES)]
    for i, b in enumerate(border):
        r = i // NCORES
        c = i % NCORES if r % 2 == 0 else NCORES - 1 - (i % NCORES)
        core_bins[c].append(b)
    # per-core: order bins by load desc; reserved empty slot = last slot of
    # last tile -> last bin must have <= 127 nodes
    for c in range(NCORES):
        core_bins[c].sort(key=lambda b: -bin_load[b])
        last = core_bins[c][-1]
        if len(bin_nodes[last]) >= 128:
            g = bin_nodes[last].pop()  # move lowest-degree node
            moved = False
            for b in core_bins[c]:
                if b != last and len(bin_nodes[b]) < 128:
                    bin_nodes[b].append(g)
                    moved = True
                    break
            if not moved:
                for c2 in range(NCORES):
                    for b in core_bins[c2]:
                        if b != last and len(bin_nodes[b]) < 128:
                            bin_nodes[b].append(g)
                            moved = True
                            break
                    if moved:
                        break
            assert moved

    core_of = np.empty(n_nodes, np.int64)
    tile_of = np.empty(n_nodes, np.int64)
    slot_of = np.empty(n_nodes, np.int64)
    for c in range(NCORES):
        for k, b in enumerate(core_bins[c]):
            for s, g in enumerate(bin_nodes[b]):
                core_of[g] = c
                tile_of[g] = k
                slot_of[g] = s
    loc_of = tile_of * 128 + slot_of

    # per (core, tile) edge counts -> uniform per-tile block counts B[t]
    ecore = core_of[dst]
    etile = tile_of[dst]
    key = ecore * tiles + etile
    counts = np.bincount(key, minlength=nbins).reshape(NCORES, tiles)
    B = np.maximum(1, (counts.max(axis=0) + 127) // 128).astype(np.int64)
    colofs = np.concatenate([[0], np.cumsum(B)])
    NB = int(colofs[-1])

    # per-core edge slot assignment: edges of (c, t) fill slots
    # colofs[t]*128 .. colofs[t]*128+cnt-1 in arbitrary (stable) order
    eorder = np.argsort(key, kind="stable")
    sk = key[eorder]
    starts = np.concatenate([[0], np.cumsum(np.bincount(sk, minlength=nbins))])
    pos = np.arange(len(sk)) - starts[sk]
    e_c = sk // tiles
    e_t = sk % tiles
    eslot = colofs[e_t] * 128 + pos          # slot within the core edge array

    nslots = NB * 128
    esrc = np.zeros((NCORES, nslots), np.int64)      # pad -> node 0
    edslot = np.full((NCORES, nslots), -1, np.int64)  # pad -> -1
    esrc[e_c, eslot] = src[eorder]
    edslot[e_c, eslot] = slot_of[dst[eorder]]

    n_empty = np.array(
        [nloc - sum(len(bin_nodes[b]) for b in core_bins[c]) for c in range(NCORES)],
        np.float32,
    )
    return dict(
        tiles=tiles, nloc=nloc, NB=NB, B=B, colofs=colofs,
        core_of=core_of, loc_of=loc_of,
        esrc=esrc, edslot=edslot, n_empty=n_empty,
    )


def _node_blocks(nloc):
    out = []
    o = 0
    while o < nloc:
        w = min(512, nloc - o)
        out.append((o, w))
        o += w
    return out


def _reg_const(nc, val, dtype=F32):
    t = nc.alloc_sbuf_tensor(f"constap-{val}", [128, 1], dtype)
    nc.gpsimd.memset(t.ap(), val)
    nc.const_aps.aps[(dtype, val)] = t.ap()


def _act_recip(nc, out_ap, in_ap):
    """ACT-engine reciprocal (~1e-5 rel err, plenty for 2e-2 tol; frees the
    DVE from its 0.8us/tile microcoded reciprocal)."""
    eng = nc.scalar
    ins = [eng.lower_ap(in_ap)]
    for arg in (0.0, 1.0, 0.0):  # bias, scale, alpha
        ins.append(mybir.ImmediateValue(dtype=F32, value=arg))
    return eng.add_instruction(
        mybir.InstActivation(
            name=nc.get_next_instruction_name(),
            func=AF.Reciprocal,
            ins=ins,
            outs=[eng.lower_ap(out_ap)],
        ))


# --------------------------------------------------------------------------
# kernel build
# --------------------------------------------------------------------------

def _build(plan, n_real):
    tiles, nloc = plan["tiles"], plan["nloc"]
    NB, B, colofs = plan["NB"], plan["B"], plan["colofs"]
    NE = NB * 128
    blocks = _node_blocks(nloc)
    nblk = len(blocks)
    inv_n = 1.0 / float(n_real)

    nc = bacc.Bacc("TRN2", target_bir_lowering=False, debug=False,
                   num_devices=NCORES)
    for v in (EPS_BN, DEN_EPS):
        _reg_const(nc, v)
    nc.all_engine_barrier()

    xT_d = nc.declare_dram_parameter("xT", [128, nloc], F32, isOutput=False)
    xg_d = nc.declare_dram_parameter("xg", [128, NE], BF16, isOutput=False)
    sm_d = nc.declare_dram_parameter("sm", [128, NE], mybir.dt.float8e4,
                                     isOutput=False)
    w1_d = nc.declare_dram_parameter("w1", [128, H], BF16, isOutput=False)
    w2_d = nc.declare_dram_parameter("w2", [128, 2 * H], BF16, isOutput=False)
    w3_d = nc.declare_dram_parameter("w3", [128, H], BF16, isOutput=False)
    we_d = nc.declare_dram_parameter("we", [128, C], BF16, isOutput=False)
    pc_d = nc.declare_dram_parameter("pcols", [128, 17], F32, isOutput=False)
    out_d = nc.declare_dram_parameter("outT", [128, nloc], F32, isOutput=True)

    with ExitStack() as ctx:
        tc = ctx.enter_context(tile.TileContext(nc))
        cst = ctx.enter_context(tc.tile_pool(name="cst", bufs=1))
        big = ctx.enter_context(tc.tile_pool(name="big", bufs=1))
        wk = ctx.enter_context(tc.tile_pool(name="wk", bufs=4))
        ps = ctx.enter_context(tc.tile_pool(name="ps", bufs=2, space="PSUM"))
        dr = ctx.enter_context(tc.tile_pool(name="dr", bufs=1, space="DRAM"))

        nc.gpsimd.load_library(library_config.standard)

        # ---- load constants / params
        xT = big.tile([128, nloc], F32, name="xT", tag="slot_xT")
        nc.sync.dma_start(xT[:], xT_d[:])
        w1 = cst.tile([128, H], BF16, name="w1")
        nc.sync.dma_start(w1[:], w1_d[:])
        w2 = cst.tile([128, 2 * H], BF16, name="w2")
        nc.sync.dma_start(w2[:], w2_d[:])
        w3 = cst.tile([128, H], BF16, name="w3")
        nc.sync.dma_start(w3[:], w3_d[:])
        we = cst.tile([128, C], BF16, name="we")
        nc.sync.dma_start(we[:], we_d[:])
        pc = cst.tile([128, 17], F32, name="pc")
        nc.sync.dma_start(pc[:], pc_d[:])
        ident = cst.tile([128, 128], F32, name="ident")
        make_identity(nc, ident[:])
        # eps-injection for the scatter matmul: psum init = epsI^T @ ones256
        # adds DEN_EPS to every den/num column so den >= eps by construction
        # (keeps the batched ACT reciprocal in range; no per-tile DVE max)
        epsI = cst.tile([128, 128], BF16, name="epsI")
        nc.scalar.activation(epsI[:], ident[:], AF.Identity, scale=DEN_EPS)
        ones256 = cst.tile([128, 256], BF16, name="ones256")
        nc.vector.memset(ones256[:], 1.0)


        g0, b0 = pc[:, 0:1], pc[:, 1:2]
        b1 = [pc[:, 2:3], pc[:, 3:4]]
        g1 = [pc[:, 4:5], pc[:, 5:6]]
        be1 = [pc[:, 6:7], pc[:, 7:8]]
        b2 = [pc[:, 8:9], pc[:, 9:10]]
        g2 = [pc[:, 10:11], pc[:, 11:12]]
        be2 = [pc[:, 12:13], pc[:, 13:14]]
        b3, be_enc, n_emp = pc[:, 14:15], pc[:, 15:16], pc[:, 16:17]

        # ---- helper: BN params from allreduced [sum, sumsq] cols
        def bn_params(st_sum, st_ssq, g_ap, beta_ap, name):
            mean = cst.tile([128, 1], F32, name=f"{name}_mean")
            nc.vector.tensor_scalar(out=mean[:], in0=st_sum, scalar1=inv_n,
                                    scalar2=None, op0=OP.mult)
            msq = cst.tile([128, 1], F32, name=f"{name}_msq")
            nc.vector.tensor_scalar(out=msq[:], in0=st_ssq, scalar1=inv_n,
                                    scalar2=None, op0=OP.mult)
            var = cst.tile([128, 1], F32, name=f"{name}_var")
            nc.vector.tensor_tensor(out=var[:], in0=mean[:], in1=mean[:],
                                    op=OP.mult)
            nc.vector.tensor_tensor(out=var[:], in0=msq[:], in1=var[:],
                                    op=OP.subtract)
            sd = cst.tile([128, 1], F32, name=f"{name}_sd")
            nc.scalar.activation(sd[:], var[:], AF.Sqrt, bias=EPS_BN)
            rs = cst.tile([128, 1], F32, name=f"{name}_rs")
            nc.vector.reciprocal(rs[:], sd[:])
            scale = cst.tile([128, 1], F32, name=f"{name}_scale")
            nc.vector.tensor_tensor(out=scale[:], in0=g_ap, in1=rs[:], op=OP.mult)
            shift = cst.tile([128, 1], F32, name=f"{name}_shift")
            nc.vector.tensor_tensor(out=shift[:], in0=mean[:], in1=scale[:],
                                    op=OP.mult)
            nc.vector.tensor_tensor(out=shift[:], in0=beta_ap, in1=shift[:],
                                    op=OP.subtract)
            return scale, shift

        def allreduce(sb_in_ap, width, name):
            bi = dr.tile([128, width], F32, name=f"{name}_in")
            bo = dr.tile([128, width], F32, name=f"{name}_out")
            nc.sync.dma_start(bi[:], sb_in_ap)
            nc.gpsimd.collective_compute(
                "AllReduce", OP.add, replica_groups=[list(range(NCORES))],
                ins=[bi[:].opt()], outs=[bo[:].opt()],
            )
            # readback on the ACT queue: a readback on the sync queue blocks
            # all later dma_starts (edge prefetches) until the CC completes
            st = cst.tile([128, width], F32, name=f"{name}_st")
            nc.scalar.dma_start(st[:], bo[:])
            return st

        # rec: batched reciprocal of all den cols lives here after the edge
        # loop (exactly tiles*128 == nloc cols); also the per-block square
        # discard target for MLP stats
        scr = big.tile([128, nloc], BF16, name="scr", tag="slotS")
        # per-tile [den | num] psum evacuation (bf16)
        denum = big.tile([128, tiles, 256], BF16, name="denum", tag="slotDN")

        # ---- phase 0: dummy 1-col AllReduce to absorb the ~35us CC-engine
        # init while xT loads and BN0 stats compute (AR0 then runs warm)
        warm = cst.tile([128, 1], F32, name="ccwarm")
        nc.vector.memset(warm[:], 0.0)
        allreduce(warm[:], 1, "warmup")

        # ---- phase 1: BN0 stats over x (empty slots are exact zeros)
        st0in = cst.tile([128, 2], F32, name="st0in")
        nc.vector.tensor_reduce(st0in[:, 0:1], xT[:],
                                axis=mybir.AxisListType.X, op=OP.add)
        st0sq = cst.tile([128, nblk], F32, name="st0sq")
        for i, (o, w) in enumerate(blocks):
            sq = wk.tile([128, 512], BF16, name="bn0sq", tag="sqscr", bufs=2)
            nc.scalar.activation(sq[:, :w], xT[:, o:o + w], AF.Square,
                                 accum_out=st0sq[:, i:i + 1])
        nc.vector.tensor_reduce(st0in[:, 1:2], st0sq[:],
                                axis=mybir.AxisListType.X, op=OP.add)
        st0 = allreduce(st0in[:], 2, "ar0")
        scale0, shift0 = bn_params(st0[:, 0:1], st0[:, 1:2], g0, b0, "bn0")

        # ---- phase 2: h (channel-major) + partition-broadcast of s0/t0
        h_cm = big.tile([128, nloc], BF16, name="h_cm", tag="slotH")
        nc.scalar.activation(h_cm[:], xT[:], AF.Relu,
                             bias=shift0[:, 0:1], scale=scale0[:, 0:1])

        # broadcast scale0/shift0 along partitions: transpose + K=1 matmul
        ones1 = cst.tile([1, 128], F32, name="ones1")
        nc.vector.memset(ones1[:], 1.0)
        stp = cst.tile([128, 128], F32, name="stp")
        nc.vector.memset(stp[:], 0.0)
        nc.vector.tensor_copy(stp[:, 0:1], scale0[:])
        nc.vector.tensor_copy(stp[:, 32:33], shift0[:])
        trp0 = ps.tile([128, 128], F32, name="trp0", tag="trp", bufs=1)
        nc.tensor.transpose(trp0[:], stp[:], ident[:])
        srow_s = cst.tile([1, 128], F32, name="srow_s")
        nc.scalar.copy(srow_s[:], trp0[0:1, :])
        srow_t = cst.tile([1, 128], F32, name="srow_t")
        nc.scalar.copy(srow_t[:], trp0[32:33, :])
        psbc = ps.tile([128, 256], F32, name="psbc", tag="edge_psum", bufs=2)
        nc.tensor.matmul(psbc[:, 0:128], lhsT=ones1[:], rhs=srow_s[:],
                         start=True, stop=True)
        nc.tensor.matmul(psbc[:, 128:256], lhsT=ones1[:], rhs=srow_t[:],
                         start=True, stop=True)
        sbc = cst.tile([128, 256], BF16, name="sbc")
        nc.scalar.copy(sbc[:], psbc[:])
        # materialize block-replicated copies so per-tile ops use plain APs
        # (broadcast reads measured ~4x slower on DVE/GPSIMD)
        BMAX = int(B.max())
        s_rep = cst.tile([128, BMAX, 128], BF16, name="s_rep")
        nc.scalar.activation(
            s_rep[:],
            sbc[:, 0:128].rearrange("p (g c) -> p g c", g=1)
            .to_broadcast([128, BMAX, 128]), AF.Identity)
        t_rep = cst.tile([128, BMAX, 128], BF16, name="t_rep")
        nc.scalar.activation(
            t_rep[:],
            sbc[:, 128:256].rearrange("p (g c) -> p g c", g=1)
            .to_broadcast([128, BMAX, 128]), AF.Identity)

        # ---- phase 3a: edge aggregation into [den|num] per tile
        z0 = big.tile([128, nloc], BF16, name="z0", tag="slotB")
        for t in range(tiles):
            bt = int(B[t])
            ncols = bt * 128
            cb = int(colofs[t]) * 128
            xg = wk.tile([128, ncols], BF16, name="xg", tag="xg", bufs=3)
            nc.sync.dma_start(xg[:], xg_d[:, cb:cb + ncols])
            sm = wk.tile([128, ncols], mybir.dt.float8e4, name="sm",
                         tag="sm", bufs=4)
            nc.sync.dma_start(sm[:], sm_d[:, cb:cb + ncols])

            # hb = relu(s*xg + t): separate tiles + flat 2D APs (in-place,
            # 3D-AP, scalar_tensor_tensor, and DMA-transpose variants all
            # measured slower)
            y = wk.tile([128, ncols], BF16, name="y", tag="y", bufs=2)
            nc.vector.tensor_tensor(out=y[:], in0=xg[:],
                                    in1=s_rep[:, 0:bt, :], op=OP.mult)
            y2 = wk.tile([128, ncols], BF16, name="y2", tag="y2", bufs=2)
            nc.vector.tensor_tensor(out=y2[:], in0=y[:],
                                    in1=t_rep[:, 0:bt, :], op=OP.add)
            hb = wk.tile([128, ncols], BF16, name="hb", tag="hb", bufs=2)
            nc.vector.tensor_scalar(out=hb[:], in0=y2[:], scalar1=0.0,
                                    scalar2=None, op0=OP.max)
            # split-plane V: first ncols = exp(hb), second = hb*exp(hb)
            V = wk.tile([128, 2 * ncols], BF16, name="V", tag="V", bufs=3)
            nc.scalar.activation(V[:, 0:ncols], hb[:], AF.Exp)
            nc.vector.tensor_tensor(out=V[:, ncols:2 * ncols], in0=hb[:],
                                    in1=V[:, 0:ncols], op=OP.mult)
            V4 = V[:].rearrange("p (two b c) -> p two b c", two=2, c=128)
            sm3 = sm[:].rearrange("p (b c) -> p b c", c=128)
            psum = ps.tile([128, 256], F32, name="epsum", tag="edge_psum",
                           bufs=2)
            nc.tensor.matmul(psum[:], lhsT=epsI[:], rhs=ones256[:],
                             start=True, stop=False)
            for b in range(bt):
                nc.tensor.matmul(psum[:],
                                 lhsT=sm3[:, b, :],
                                 rhs=V4[:, :, b, :],
                                 start=False, stop=(b == bt - 1))
            nc.scalar.copy(denum[:, t, :], psum[:])

        # ---- phase 3b: one batched reciprocal (single ACT table load),
        # then per-tile agg = num*rec, transpose, z0 = aggT + h
        _act_recip(nc, scr[:].rearrange("p (t c) -> p t c", c=128),
                   denum[:, :, 0:128])
        for t in range(tiles):
            agg = wk.tile([128, 128], F32, name="agg", tag="agg", bufs=3)
            nc.vector.tensor_tensor(out=agg[:], in0=denum[:, t, 128:256],
                                    in1=scr[:, t * 128:(t + 1) * 128],
                                    op=OP.mult)
            trp2 = ps.tile([128, 128], F32, name="trp_a", tag="trp", bufs=1)
            nc.tensor.transpose(trp2[:], agg[:], ident[:])
            nc.vector.tensor_tensor(
                out=z0[:, t * 128:(t + 1) * 128], in0=trp2[:],
                in1=h_cm[:, t * 128:(t + 1) * 128], op=OP.add)

        # ---- phases 4..6: MLP layer helper
        def mlp_layer(zin_list, wtile, wofs, kparts, zout_tags, b_aps, g_aps,
                      be_aps, name):
            """zout = relu(BN(sum_k W[k]^T @ zin[k] + b)) with empty-slot
            corrected global BN. zraw halves bf16; zout bf16."""
            mparts = len(zout_tags)
            zraw = [big.tile([128, nloc], BF16, name=f"{name}_raw{m}",
                             tag=zout_tags[m][0]) for m in range(mparts)]
            ssum = [cst.tile([128, nblk], F32, name=f"{name}_ss{m}")
                    for m in range(mparts)]
            ssq = [cst.tile([128, nblk], F32, name=f"{name}_sq{m}")
                   for m in range(mparts)]
            for i, (o, w) in enumerate(blocks):
                for m in range(mparts):
                    pmm = ps.tile([128, 512], F32, name=f"{name}_ps",
                                  tag="mm", bufs=3)
                    for k in range(kparts):
                        lhs = wtile[:, wofs(k, m):wofs(k, m) + 128]
                        nc.tensor.matmul(
                            pmm[:, :w], lhsT=lhs,
                            rhs=zin_list[k][:, o:o + w],
                            start=(k == 0), stop=(k == kparts - 1))
                    # p1 (bias + row sums): split halves ACT/DVE
                    if m == 0:
                        nc.scalar.activation(zraw[m][:, o:o + w], pmm[:, :w],
                                             AF.Identity, bias=b_aps[m],
                                             accum_out=ssum[m][:, i:i + 1])
                    else:
                        nc.vector.tensor_scalar(out=zraw[m][:, o:o + w],
                                                in0=pmm[:, :w],
                                                scalar1=b_aps[m], scalar2=0.0,
                                                op0=OP.add, op1=OP.add,
                                                accum_out=ssum[m][:, i:i + 1])
                    nc.scalar.activation(scr[:, o:o + w], pmm[:, :w],
                                         AF.Square, bias=b_aps[m],
                                         accum_out=ssq[m][:, i:i + 1])
            # stats with empty-slot correction (reserved last slot is empty)
            arin = cst.tile([128, 2 * mparts], F32, name=f"{name}_arin")
            for m in range(mparts):
                rs_ = cst.tile([128, 2], F32, name=f"{name}_r{m}")
                nc.vector.tensor_reduce(rs_[:, 0:1], ssum[m][:],
                                        axis=mybir.AxisListType.X, op=OP.add)
                nc.vector.tensor_reduce(rs_[:, 1:2], ssq[m][:],
                                        axis=mybir.AxisListType.X, op=OP.add)
                delta = cst.tile([128, 1], F32, name=f"{name}_dl{m}")
                nc.vector.tensor_copy(delta[:], zraw[m][:, nloc - 1:nloc])
                t1 = cst.tile([128, 1], F32, name=f"{name}_t1{m}")
                nc.vector.tensor_tensor(out=t1[:], in0=delta[:], in1=n_emp,
                                        op=OP.mult)
                nc.vector.tensor_tensor(out=arin[:, m:m + 1], in0=rs_[:, 0:1],
                                        in1=t1[:], op=OP.subtract)
                d2 = cst.tile([128, 1], F32, name=f"{name}_d2{m}")
                nc.vector.tensor_tensor(out=d2[:], in0=delta[:], in1=t1[:],
                                        op=OP.mult)
                nc.vector.tensor_tensor(out=arin[:, mparts + m:mparts + m + 1],
                                        in0=rs_[:, 1:2], in1=d2[:],
                                        op=OP.subtract)
            st = allreduce(arin[:], 2 * mparts, f"{name}_ar")
            zout = []
            scsh = []
            for m in range(mparts):
                scsh.append(bn_params(st[:, m:m + 1],
                                      st[:, mparts + m:mparts + m + 1],
                                      g_aps[m], be_aps[m], f"{name}_p{m}"))
                zout.append(big.tile([128, nloc], BF16, name=f"{name}_n{m}",
                                     tag=zout_tags[m][1]))
            for o, w in blocks:
                for m in range(mparts):
                    sc, sh = scsh[m]
                    if m == 0:
                        nc.scalar.activation(zout[m][:, o:o + w],
                                             zraw[m][:, o:o + w], AF.Relu,
                                             bias=sh[:, 0:1], scale=sc[:, 0:1])
                    else:
                        # relu(zraw*sc+sh) as two fast tensor_scalar passes
                        tmp = wk.tile([128, 512], BF16, name=f"{name}_rt",
                                      tag="sqscr", bufs=2)
                        nc.vector.tensor_scalar(out=tmp[:, :w],
                                                in0=zraw[m][:, o:o + w],
                                                scalar1=sc[:, 0:1],
                                                scalar2=sh[:, 0:1],
                                                op0=OP.mult, op1=OP.add)
                        nc.vector.tensor_scalar(out=zout[m][:, o:o + w],
                                                in0=tmp[:, :w], scalar1=0.0,
                                                scalar2=None, op0=OP.max)
            return zout

        # L1: z0 [C,n] -> z1 halves; W1 [128, 2H]: lhsT for half m = w1[:, m*128...]
        z1 = mlp_layer([z0], w1, lambda k, m: m * 128, 1,
                       [("slotA", "slotB"), ("slotC", "slotD")],
                       b1, g1, be1, "l1")
        # L2: z1 (2 K-parts) -> z2 halves; W2sb [128, 512]:
        # lhsT(k,m) at col k*256 + m*128
        z2 = mlp_layer(z1, w2, lambda k, m: k * 256 + m * 128, 2,
                       [("slotA", "slotB"), ("slotC", "slotD")],
                       b2, g2, be2, "l2")

        # ---- phase 7: z3 = W3^T @ z2 + b3; u = z3 + xT; out = We^T @ u + be
        for o, w in blocks:
            ps3 = ps.tile([128, 512], F32, name="ps3", tag="mm", bufs=3)
            for k in range(2):
                nc.tensor.matmul(
                    ps3[:, :w], lhsT=w3[:, k * 128:k * 128 + 128],
                    rhs=z2[k][:, o:o + w],
                    start=(k == 0), stop=(k == 1))
            # u2 = (ps3 + b3) + xT fused on DVE (frees ACT for L2 relus)
            u2 = wk.tile([128, 512], BF16, name="u2", tag="u2", bufs=3)
            nc.vector.scalar_tensor_tensor(
                out=u2[:, :w], in0=ps3[:, :w], scalar=b3,
                in1=xT[:, o:o + w], op0=OP.add, op1=OP.add)
            ps4 = ps.tile([128, 512], F32, name="ps4", tag="mm2", bufs=2)
            nc.tensor.matmul(ps4[:, :w], lhsT=we[:], rhs=u2[:, :w],
                             start=True, stop=True)
            ob = wk.tile([128, 512], F32, name="ob", tag="ob", bufs=2)
            nc.vector.tensor_scalar(out=ob[:, :w], in0=ps4[:, :w],
                                    scalar1=be_enc, scalar2=0.0,
                                    op0=OP.add, op1=OP.add)
            nc.sync.dma_start(out_d[:, o:o + w], ob[:, :w])

    nc.compile()
    return nc


# --------------------------------------------------------------------------
# public entry
# --------------------------------------------------------------------------

_CACHE = {}
LAST_RESULT = None


def _run(x, edge_index, bn_g, bn_b, W1, b1, g1, be1, W2, b2, g2, be2,
         W3, b3, We, be, n_nodes):
    import ml_dtypes
    bf16 = ml_dtypes.bfloat16

    src = np.asarray(edge_index[0], dtype=np.int64)
    dst = np.asarray(edge_index[1], dtype=np.int64)
    x = np.asarray(x, dtype=np.float32)

    plan = _plan(n_nodes, src, dst)
    tiles, nloc, NB = plan["tiles"], plan["nloc"], plan["NB"]

    key = (tiles, NB, tuple(plan["B"].tolist()), n_nodes)
    if key not in _CACHE:
        _CACHE[key] = _build(plan, n_nodes)
    nc = _CACHE[key]

    h_dim = W1.shape[1]
    # pack weights
    w2sb = np.ascontiguousarray(
        W2.reshape(2, 128, h_dim).transpose(1, 0, 2).reshape(128, 2 * h_dim)
    ).astype(bf16)
    w3sb = np.ascontiguousarray(
        W3.reshape(2, 128, 128).transpose(1, 0, 2).reshape(128, 256)
    ).astype(bf16)

    halves = lambda v: [np.asarray(v[:128], np.float32),
                        np.asarray(v[128:], np.float32)]
    b1h, g1h, be1h = halves(b1), halves(g1), halves(be1)
    b2h, g2h, be2h = halves(b2), halves(g2), halves(be2)

    nslots = NB * 128

    in_maps = []
    core_nodes = []
    for c in range(NCORES):
        sel = plan["core_of"] == c
        nodes_c = np.nonzero(sel)[0]
        core_nodes.append(nodes_c)
        xr = np.zeros((nloc, 128), np.float32)
        xr[plan["loc_of"][nodes_c]] = x[nodes_c]

        # host pre-gather (pure indexing): edge-major x[src] tiles, bf16.
        # edge slot e = b*128 + lane -> SBUF [lane, b, ch]
        esrc = plan["esrc"][c]
        xe = x[esrc].astype(bf16)                            # [nslots, 128]
        xg = np.ascontiguousarray(
            xe.reshape(NB, 128, 128).transpose(1, 0, 2).reshape(128, nslots))

        # one-hot scatter matrix S: [lane, block, dst_slot]
        ed = plan["edslot"][c]
        S2 = np.zeros((nslots, 128), np.float32)
        valid = np.nonzero(ed >= 0)[0]
        S2[valid, ed[valid]] = 1.0
        Sm = np.ascontiguousarray(
            S2.reshape(NB, 128, 128).transpose(1, 0, 2).reshape(128, nslots)
        ).astype(ml_dtypes.float8_e4m3)

        pcols = np.zeros((128, 17), np.float32)
        pcols[:, 0] = bn_g
        pcols[:, 1] = bn_b
        for i2 in range(2):
            pcols[:, 2 + i2] = b1h[i2]
            pcols[:, 4 + i2] = g1h[i2]
            pcols[:, 6 + i2] = be1h[i2]
            pcols[:, 8 + i2] = b2h[i2]
            pcols[:, 10 + i2] = g2h[i2]
            pcols[:, 12 + i2] = be2h[i2]
        pcols[:, 14] = b3
        pcols[:, 15] = be
        pcols[:, 16] = plan["n_empty"][c]
        in_maps.append({
            "xT": np.ascontiguousarray(xr.T),
            "xg": xg,
            "sm": Sm,
            "w1": np.asarray(W1, np.float32).astype(bf16),
            "w2": w2sb,
            "w3": w3sb,
            "we": np.asarray(We, np.float32).astype(bf16),
            "pcols": pcols,
        })

    import os
    trace = bool(os.environ.get("KTRACE"))
    res = run_bass_kernel_spmd(nc, in_maps, list(range(NCORES)), trace=trace)
    global LAST_RESULT
    LAST_RESULT = res
    out = np.empty((n_nodes, 128), np.float32)
    for c in range(NCORES):
        nodes_c = core_nodes[c]
        out[nodes_c] = res.results[c]["outT"][:, plan["loc_of"][nodes_c]].T
    return out


def kernel(x, edge_index, bn_g, bn_b, W1, b1, g1, be1, W2, b2, g2, be2,
           W3, b3, We, be):
    return _run(x, edge_index, bn_g, bn_b, W1, b1, g1, be1, W2, b2, g2, be2,
                W3, b3, We, be, n_nodes=x.shape[0])


# revision 66
# speedup vs baseline: 1.0175x; 1.0175x over previous
"""Trainium2 Bass kernel for DeepGCNLayer(GENConv softmax-aggr) + encoder.

Computation (see reference):
  h  = relu(batchnorm(x))                       # BN0 over all N nodes
  msg_e = relu(h[src_e]) + eps = h[src_e] + eps # h >= 0 already
  agg_v = softmax-weighted mean of msg over incoming edges (t=1)
  z0 = agg + h
  z1 = relu(BN1(z0 @ W1 + b1)); z2 = relu(BN2(z1 @ W2 + b2))
  out = (x + z2 @ W3 + b3) @ We + be

Strategy (8 NeuronCores, SPMD single program):
  * Host packs nodes into 128-slot tiles balanced by in-degree; edges are
    assigned to the core owning their dst node and packed into per-tile
    128-edge blocks.  The host PRE-GATHERS x[src] rows into edge-major
    bf16 tiles (pure indexing) and prebuilds the one-hot scatter matrix
    S (edge-slot -> dst-slot), so the device never does a per-edge
    gather (the previous dma_gather dominated at ~745us of GpSimd time)
    and needs no AllGather at all.
  * Device: BN0 stats via fused DVE reduce ops + tiny AllReduce; the
    per-channel scale/shift are broadcast across partitions with a
    transpose + K=1 ones-matmul so the per-edge affine can run in
    edge-major layout (DVE/GPSIMD tensor_tensor with free-dim broadcast).
  * Per node tile: y=xg*s (DVE), y2=y+t (GPSIMD), hb=relu (DVE),
    V=[exp(hb) | hb*exp(hb)] (ACT+DVE), then 12 chained PE matmuls
    S_b^T @ V_b accumulate [den|num] per dst slot; DVE recip+mult and a
    PE transpose produce agg, added to h for z0.
  * MLP: bf16 weight-stationary PE matmuls over 512-col blocks; BN sums
    via ACT accum_out, sum-of-squares via fused tensor_tensor_reduce;
    empty-slot corrected global BN via two more tiny AllReduces.
"""

import math
import numpy as np
from contextlib import ExitStack

import concourse.bass as bass
import concourse.tile as tile
from concourse import bacc, mybir, library_config
from concourse.bass_utils import run_bass_kernel_spmd
from concourse.masks import make_identity

F32 = mybir.dt.float32
BF16 = mybir.dt.bfloat16
AF = mybir.ActivationFunctionType
OP = mybir.AluOpType

NCORES = 8
EPS_BN = 1e-5
# den >= 1 for any slot with an incoming edge (each edge contributes
# exp(hb) >= 1), so this guard only fires on empty slots; keep it above the
# scalar-engine reciprocal's 2^-42 domain floor.
DEN_EPS = 1e-6

# full-size problem config (hardcoded; harness calls kernel() directly)
N_FULL = 50000
C = 128
H = 256


# --------------------------------------------------------------------------
# host-side graph partitioning (indexing only, no arithmetic on values)
# --------------------------------------------------------------------------

def _plan(n_nodes, src, dst):
    """Pack nodes into 128-slot bins balanced by in-degree; assign bins to
    cores; per (core, tile) edge blocks with slot assignments."""
    import heapq

    tiles = math.ceil(n_nodes / (NCORES * 128) + 0.02)  # slack for reserve
    if tiles * NCORES * 128 - NCORES < n_nodes:
        tiles += 1
    nbins = NCORES * tiles
    nloc = tiles * 128

    deg = np.bincount(dst, minlength=n_nodes).astype(np.int64)
    order = np.argsort(-deg, kind="stable")
    heap = [(0, i) for i in range(nbins)]
    heapq.heapify(heap)
    bin_nodes = [[] for _ in range(nbins)]
    bin_load = np.zeros(nbins, np.int64)
    for g in order:
        d = int(deg[g])
        spill = []
        while True:
            load, b = heapq.heappop(heap)
            if len(bin_nodes[b]) < 128:
                bin_nodes[b].append(g)
                bin_load[b] = load + d
                heapq.heappush(heap, (load + d, b))
                break
            spill.append((load, b))
        for it in spill:
            heapq.heappush(heap, it)

    # snake-assign bins to cores by load
    border = np.argsort(-bin_load, kind="stable")
    core_bins = [[] for _ in range(NCORES)]
    for i, b in enumerate(border):
        r = i // NCORES
        c = i % NCORES if r % 2 == 0 else NCORES - 1 - (i % NCORES)
        core_bins[c].append(b)
    # per-core: order bins by load desc; reserved empty slot = last slot of
    # last tile -> last bin must have <= 127 nodes
    for c in range(NCORES):
        core_bins[c].sort(key=lambda b: -bin_load[b])
        last = core_bins[c][-1]
        if len(bin_nodes[last]) >= 128:
            g = bin_nodes[last].pop()  # move lowest-degree node
            moved = False
            for b in core_bins[c]:
                if b != last and len(bin_nodes[b]) < 128:
                    bin_nodes[b].append(g)
                    moved = True
                    break
            if not moved:
                for c2 in range(NCORES):
                    for b in core_bins[c2]:
                        if b != last and len(bin_nodes[b]) < 128:
                            bin_nodes[b].append(g)
                            moved = True
                            break
                    if moved:
                        break
            assert moved

    core_of = np.empty(n_nodes, np.int64)
    tile_of = np.empty(n_nodes, np.int64)
    slot_of = np.empty(n_nodes, np.int64)
    for c in range(NCORES):
        for k, b in enumerate(core_bins[c]):
            for s, g in enumerate(bin_nodes[b]):
                core_of[g] = c
                tile_of[g] = k
                slot_of[g] = s
    loc_of = tile_of * 128 + slot_of

    # per (core, tile) edge counts -> uniform per-tile block counts B[t]
    ecore = core_of[dst]
    etile = tile_of[dst]
    key = ecore * tiles + etile
    counts = np.bincount(key, minlength=nbins).reshape(NCORES, tiles)
    B = np.maximum(1, (counts.max(axis=0) + 127) // 128).astype(np.int64)
    colofs = np.concatenate([[0], np.cumsum(B)])
    NB = int(colofs[-1])

    # per-core edge slot assignment: edges of (c, t) fill slots
    # colofs[t]*128 .. colofs[t]*128+cnt-1 in arbitrary (stable) order
    eorder = np.argsort(key, kind="stable")
    sk = key[eorder]
    starts = np.concatenate([[0], np.cumsum(np.bincount(sk, minlength=nbins))])
    pos = np.arange(len(sk)) - starts[sk]
    e_c = sk // tiles
    e_t = sk % tiles
    eslot = colofs[e_t] * 128 + pos          # slot within the core edge array

    nslots = NB * 128
    esrc = np.zeros((NCORES, nslots), np.int64)      # pad -> node 0
    edslot = np.full((NCORES, nslots), -1, np.int64)  # pad -> -1
    esrc[e_c, eslot] = src[eorder]
    edslot[e_c, eslot] = slot_of[dst[eorder]]

    n_empty = np.array(
        [nloc - sum(len(bin_nodes[b]) for b in core_bins[c]) for c in range(NCORES)],
        np.float32,
    )
    return dict(
        tiles=tiles, nloc=nloc, NB=NB, B=B, colofs=colofs,
        core_of=core_of, loc_of=loc_of,
        esrc=esrc, edslot=edslot, n_empty=n_empty,
    )


def _node_blocks(nloc):
    out = []
    o = 0
    while o < nloc:
        w = min(512, nloc - o)
        out.append((o, w))
        o += w
    return out


def _reg_const(nc, val, dtype=F32):
    t = nc.alloc_sbuf_tensor(f"constap-{val}", [128, 1], dtype)
    nc.gpsimd.memset(t.ap(), val)
    nc.const_aps.aps[(dtype, val)] = t.ap()


def _act_recip(nc, out_ap, in_ap):
    """ACT-engine reciprocal (~1e-5 rel err, plenty for 2e-2 tol; frees the
    DVE from its 0.8us/tile microcoded reciprocal)."""
    eng = nc.scalar
    ins = [eng.lower_ap(in_ap)]
    for arg in (0.0, 1.0, 0.0):  # bias, scale, alpha
        ins.append(mybir.ImmediateValue(dtype=F32, value=arg))
    return eng.add_instruction(
        mybir.InstActivation(
            name=nc.get_next_instruction_name(),
            func=AF.Reciprocal,
            ins=ins,
            outs=[eng.lower_ap(out_ap)],
        ))


# --------------------------------------------------------------------------
# kernel build
# --------------------------------------------------------------------------

def _build(plan, n_real):
    tiles, nloc = plan["tiles"], plan["nloc"]
    NB, B, colofs = plan["NB"], plan["B"], plan["colofs"]
    NE = NB * 128
    blocks = _node_blocks(nloc)
    nblk = len(blocks)
    inv_n = 1.0 / float(n_real)

    nc = bacc.Bacc("TRN2", target_bir_lowering=False, debug=False,
                   num_devices=NCORES)
    for v in (EPS_BN, DEN_EPS):
        _reg_const(nc, v)
    nc.all_engine_barrier()

    xT_d = nc.declare_dram_parameter("xT", [128, nloc], F32, isOutput=False)
    xg_d = nc.declare_dram_parameter("xg", [128, NE], BF16, isOutput=False)
    sm_d = nc.declare_dram_parameter("sm", [128, NE], mybir.dt.float8e4,
                                     isOutput=False)
    w1_d = nc.declare_dram_parameter("w1", [128, H], BF16, isOutput=False)
    w2_d = nc.declare_dram_parameter("w2", [128, 2 * H], BF16, isOutput=False)
    w3_d = nc.declare_dram_parameter("w3", [128, H], BF16, isOutput=False)
    we_d = nc.declare_dram_parameter("we", [128, C], BF16, isOutput=False)
    pc_d = nc.declare_dram_parameter("pcols", [128, 17], F32, isOutput=False)
    out_d = nc.declare_dram_parameter("outT", [128, nloc], F32, isOutput=True)

    with ExitStack() as ctx:
        tc = ctx.enter_context(tile.TileContext(nc))
        cst = ctx.enter_context(tc.tile_pool(name="cst", bufs=1))
        big = ctx.enter_context(tc.tile_pool(name="big", bufs=1))
        wk = ctx.enter_context(tc.tile_pool(name="wk", bufs=4))
        ps = ctx.enter_context(tc.tile_pool(name="ps", bufs=2, space="PSUM"))
        dr = ctx.enter_context(tc.tile_pool(name="dr", bufs=1, space="DRAM"))

        nc.gpsimd.load_library(library_config.standard)

        # ---- load constants / params
        xT = big.tile([128, nloc], F32, name="xT", tag="slot_xT")
        nc.sync.dma_start(xT[:], xT_d[:])
        w1 = cst.tile([128, H], BF16, name="w1")
        nc.sync.dma_start(w1[:], w1_d[:])
        w2 = cst.tile([128, 2 * H], BF16, name="w2")
        nc.sync.dma_start(w2[:], w2_d[:])
        w3 = cst.tile([128, H], BF16, name="w3")
        nc.sync.dma_start(w3[:], w3_d[:])
        we = cst.tile([128, C], BF16, name="we")
        nc.sync.dma_start(we[:], we_d[:])
        pc = cst.tile([128, 17], F32, name="pc")
        nc.sync.dma_start(pc[:], pc_d[:])
        ident = cst.tile([128, 128], F32, name="ident")
        make_identity(nc, ident[:])
        # eps-injection for the scatter matmul: psum init = epsI^T @ ones256
        # adds DEN_EPS to every den/num column so den >= eps by construction
        # (keeps the batched ACT reciprocal in range; no per-tile DVE max)
        epsI = cst.tile([128, 128], BF16, name="epsI")
        nc.scalar.activation(epsI[:], ident[:], AF.Identity, scale=DEN_EPS)
        ones256 = cst.tile([128, 256], BF16, name="ones256")
        nc.vector.memset(ones256[:], 1.0)


        g0, b0 = pc[:, 0:1], pc[:, 1:2]
        b1 = [pc[:, 2:3], pc[:, 3:4]]
        g1 = [pc[:, 4:5], pc[:, 5:6]]
        be1 = [pc[:, 6:7], pc[:, 7:8]]
        b2 = [pc[:, 8:9], pc[:, 9:10]]
        g2 = [pc[:, 10:11], pc[:, 11:12]]
        be2 = [pc[:, 12:13], pc[:, 13:14]]
        b3, be_enc, n_emp = pc[:, 14:15], pc[:, 15:16], pc[:, 16:17]

        # ---- helper: BN params from allreduced [sum, sumsq] cols
        def bn_params(st_sum, st_ssq, g_ap, beta_ap, name):
            mean = cst.tile([128, 1], F32, name=f"{name}_mean")
            nc.vector.tensor_scalar(out=mean[:], in0=st_sum, scalar1=inv_n,
                                    scalar2=None, op0=OP.mult)
            msq = cst.tile([128, 1], F32, name=f"{name}_msq")
            nc.vector.tensor_scalar(out=msq[:], in0=st_ssq, scalar1=inv_n,
                                    scalar2=None, op0=OP.mult)
            var = cst.tile([128, 1], F32, name=f"{name}_var")
            nc.vector.tensor_tensor(out=var[:], in0=mean[:], in1=mean[:],
                                    op=OP.mult)
            nc.vector.tensor_tensor(out=var[:], in0=msq[:], in1=var[:],
                                    op=OP.subtract)
            sd = cst.tile([128, 1], F32, name=f"{name}_sd")
            nc.scalar.activation(sd[:], var[:], AF.Sqrt, bias=EPS_BN)
            rs = cst.tile([128, 1], F32, name=f"{name}_rs")
            nc.vector.reciprocal(rs[:], sd[:])
            scale = cst.tile([128, 1], F32, name=f"{name}_scale")
            nc.vector.tensor_tensor(out=scale[:], in0=g_ap, in1=rs[:], op=OP.mult)
            shift = cst.tile([128, 1], F32, name=f"{name}_shift")
            nc.vector.tensor_tensor(out=shift[:], in0=mean[:], in1=scale[:],
                                    op=OP.mult)
            nc.vector.tensor_tensor(out=shift[:], in0=beta_ap, in1=shift[:],
                                    op=OP.subtract)
            return scale, shift

        def allreduce(sb_in_ap, width, name):
            bi = dr.tile([128, width], F32, name=f"{name}_in")
            bo = dr.tile([128, width], F32, name=f"{name}_out")
            nc.sync.dma_start(bi[:], sb_in_ap)
            nc.gpsimd.collective_compute(
                "AllReduce", OP.add, replica_groups=[list(range(NCORES))],
                ins=[bi[:].opt()], outs=[bo[:].opt()],
            )
            # readback on the ACT queue: a readback on the sync queue blocks
            # all later dma_starts (edge prefetches) until the CC completes
            st = cst.tile([128, width], F32, name=f"{name}_st")
            nc.scalar.dma_start(st[:], bo[:])
            return st

        # rec: batched reciprocal of all den cols lives here after the edge
        # loop (exactly tiles*128 == nloc cols); also the per-block square
        # discard target for MLP stats
        scr = big.tile([128, nloc], BF16, name="scr", tag="slotS")
        # per-tile [den | num] psum evacuation (bf16)
        denum = big.tile([128, tiles, 256], BF16, name="denum", tag="slotDN")

        # ---- phase 0: dummy 1-col AllReduce to absorb the ~35us CC-engine
        # init while xT loads and BN0 stats compute (AR0 then runs warm)
        warm = cst.tile([128, 1], F32, name="ccwarm")
        nc.vector.memset(warm[:], 0.0)
        allreduce(warm[:], 1, "warmup")

        # ---- phase 1: BN0 stats over x (empty slots are exact zeros)
        st0in = cst.tile([128, 2], F32, name="st0in")
        nc.vector.tensor_reduce(st0in[:, 0:1], xT[:],
                                axis=mybir.AxisListType.X, op=OP.add)
        st0sq = cst.tile([128, nblk], F32, name="st0sq")
        for i, (o, w) in enumerate(blocks):
            sq = wk.tile([128, 512], BF16, name="bn0sq", tag="sqscr", bufs=2)
            nc.scalar.activation(sq[:, :w], xT[:, o:o + w], AF.Square,
                                 accum_out=st0sq[:, i:i + 1])
        nc.vector.tensor_reduce(st0in[:, 1:2], st0sq[:],
                                axis=mybir.AxisListType.X, op=OP.add)
        st0 = allreduce(st0in[:], 2, "ar0")
        scale0, shift0 = bn_params(st0[:, 0:1], st0[:, 1:2], g0, b0, "bn0")

        # ---- phase 2: h (channel-major) + partition-broadcast of s0/t0
        h_cm = big.tile([128, nloc], BF16, name="h_cm", tag="slotH")
        nc.scalar.activation(h_cm[:], xT[:], AF.Relu,
                             bias=shift0[:, 0:1], scale=scale0[:, 0:1])

        # broadcast scale0/shift0 along partitions: transpose + K=1 matmul
        ones1 = cst.tile([1, 128], F32, name="ones1")
        nc.vector.memset(ones1[:], 1.0)
        stp = cst.tile([128, 128], F32, name="stp")
        nc.vector.memset(stp[:], 0.0)
        nc.vector.tensor_copy(stp[:, 0:1], scale0[:])
        nc.vector.tensor_copy(stp[:, 32:33], shift0[:])
        trp0 = ps.tile([128, 128], F32, name="trp0", tag="trp", bufs=1)
        nc.tensor.transpose(trp0[:], stp[:], ident[:])
        srow_s = cst.tile([1, 128], F32, name="srow_s")
        nc.scalar.copy(srow_s[:], trp0[0:1, :])
        srow_t = cst.tile([1, 128], F32, name="srow_t")
        nc.scalar.copy(srow_t[:], trp0[32:33, :])
        psbc = ps.tile([128, 256], F32, name="psbc", tag="edge_psum", bufs=2)
        nc.tensor.matmul(psbc[:, 0:128], lhsT=ones1[:], rhs=srow_s[:],
                         start=True, stop=True)
        nc.tensor.matmul(psbc[:, 128:256], lhsT=ones1[:], rhs=srow_t[:],
                         start=True, stop=True)
        sbc = cst.tile([128, 256], BF16, name="sbc")
        nc.scalar.copy(sbc[:], psbc[:])
        # materialize block-replicated copies so per-tile ops use plain APs
        # (broadcast reads measured ~4x slower on DVE/GPSIMD)
        BMAX = int(B.max())
        s_rep = cst.tile([128, BMAX, 128], BF16, name="s_rep")
        nc.scalar.activation(
            s_rep[:],
            sbc[:, 0:128].rearrange("p (g c) -> p g c", g=1)
            .to_broadcast([128, BMAX, 128]), AF.Identity)
        t_rep = cst.tile([128, BMAX, 128], BF16, name="t_rep")
        nc.scalar.activation(
            t_rep[:],
            sbc[:, 128:256].rearrange("p (g c) -> p g c", g=1)
            .to_broadcast([128, BMAX, 128]), AF.Identity)

        # ---- phase 3a: edge aggregation into [den|num] per tile
        z0 = big.tile([128, nloc], BF16, name="z0", tag="slotB")
        for t in range(tiles):
            bt = int(B[t])
            ncols = bt * 128
            cb = int(colofs[t]) * 128
            xg = wk.tile([128, ncols], BF16, name="xg", tag="xg", bufs=3)
            nc.sync.dma_start(xg[:], xg_d[:, cb:cb + ncols])
            sm = wk.tile([128, ncols], mybir.dt.float8e4, name="sm",
                         tag="sm", bufs=4)
            nc.sync.dma_start(sm[:], sm_d[:, cb:cb + ncols])

            # hb = relu(s*xg + t): separate tiles + flat 2D APs (in-place,
            # 3D-AP, scalar_tensor_tensor, and DMA-transpose variants all
            # measured slower)
            y = wk.tile([128, ncols], BF16, name="y", tag="y", bufs=2)
            nc.vector.tensor_tensor(out=y[:], in0=xg[:],
                                    in1=s_rep[:, 0:bt, :], op=OP.mult)
            y2 = wk.tile([128, ncols], BF16, name="y2", tag="y2", bufs=2)
            nc.vector.tensor_tensor(out=y2[:], in0=y[:],
                                    in1=t_rep[:, 0:bt, :], op=OP.add)
            hb = wk.tile([128, ncols], BF16, name="hb", tag="hb", bufs=2)
            nc.vector.tensor_scalar(out=hb[:], in0=y2[:], scalar1=0.0,
                                    scalar2=None, op0=OP.max)
            # split-plane V: first ncols = exp(hb), second = hb*exp(hb)
            V = wk.tile([128, 2 * ncols], BF16, name="V", tag="V", bufs=3)
            nc.scalar.activation(V[:, 0:ncols], hb[:], AF.Exp)
            nc.vector.tensor_tensor(out=V[:, ncols:2 * ncols], in0=hb[:],
                                    in1=V[:, 0:ncols], op=OP.mult)
            V4 = V[:].rearrange("p (two b c) -> p two b c", two=2, c=128)
            sm3 = sm[:].rearrange("p (b c) -> p b c", c=128)
            psum = ps.tile([128, 256], F32, name="epsum", tag="edge_psum",
                           bufs=2)
            nc.tensor.matmul(psum[:], lhsT=epsI[:], rhs=ones256[:],
                             start=True, stop=False)
            for b in range(bt):
                nc.tensor.matmul(psum[:],
                                 lhsT=sm3[:, b, :],
                                 rhs=V4[:, :, b, :],
                                 start=False, stop=(b == bt - 1))
            nc.scalar.copy(denum[:, t, :], psum[:])

        # ---- phase 3b: one batched reciprocal (single ACT table load),
        # then per-tile agg = num*rec, transpose, z0 = aggT + h
        _act_recip(nc, scr[:].rearrange("p (t c) -> p t c", c=128),
                   denum[:, :, 0:128])
        for t in range(tiles):
            agg = wk.tile([128, 128], F32, name="agg", tag="agg", bufs=3)
            nc.vector.tensor_tensor(out=agg[:], in0=denum[:, t, 128:256],
                                    in1=scr[:, t * 128:(t + 1) * 128],
                                    op=OP.mult)
            trp2 = ps.tile([128, 128], F32, name="trp_a", tag="trp", bufs=1)
            nc.tensor.transpose(trp2[:], agg[:], ident[:])
            nc.vector.tensor_tensor(
                out=z0[:, t * 128:(t + 1) * 128], in0=trp2[:],
                in1=h_cm[:, t * 128:(t + 1) * 128], op=OP.add)

        # ---- phases 4..6: MLP layer helper
        def mlp_layer(zin_list, wtile, wofs, kparts, zout_tags, b_aps, g_aps,
                      be_aps, name):
            """zout = relu(BN(sum_k W[k]^T @ zin[k] + b)) with empty-slot
            corrected global BN. zraw halves bf16; zout bf16."""
            mparts = len(zout_tags)
            zraw = [big.tile([128, nloc], BF16, name=f"{name}_raw{m}",
                             tag=zout_tags[m][0]) for m in range(mparts)]
            ssum = [cst.tile([128, nblk], F32, name=f"{name}_ss{m}")
                    for m in range(mparts)]
            ssq = [cst.tile([128, nblk], F32, name=f"{name}_sq{m}")
                   for m in range(mparts)]
            for i, (o, w) in enumerate(blocks):
                for m in range(mparts):
                    pmm = ps.tile([128, 512], F32, name=f"{name}_ps",
                                  tag="mm", bufs=3)
                    for k in range(kparts):
                        lhs = wtile[:, wofs(k, m):wofs(k, m) + 128]
                        nc.tensor.matmul(
                            pmm[:, :w], lhsT=lhs,
                            rhs=zin_list[k][:, o:o + w],
                            start=(k == 0), stop=(k == kparts - 1))
                    # p1 (bias + row sums): split halves ACT/DVE
                    if m == 0:
                        nc.scalar.activation(zraw[m][:, o:o + w], pmm[:, :w],
                                             AF.Identity, bias=b_aps[m],
                                             accum_out=ssum[m][:, i:i + 1])
                    else:
                        nc.vector.tensor_scalar(out=zraw[m][:, o:o + w],
                                                in0=pmm[:, :w],
                                                scalar1=b_aps[m], scalar2=0.0,
                                                op0=OP.add, op1=OP.add,
                                                accum_out=ssum[m][:, i:i + 1])
                    nc.scalar.activation(scr[:, o:o + w], pmm[:, :w],
                                         AF.Square, bias=b_aps[m],
                                         accum_out=ssq[m][:, i:i + 1])
            # stats with empty-slot correction (reserved last slot is empty)
            arin = cst.tile([128, 2 * mparts], F32, name=f"{name}_arin")
            for m in range(mparts):
                rs_ = cst.tile([128, 2], F32, name=f"{name}_r{m}")
                nc.vector.tensor_reduce(rs_[:, 0:1], ssum[m][:],
                                        axis=mybir.AxisListType.X, op=OP.add)
                nc.vector.tensor_reduce(rs_[:, 1:2], ssq[m][:],
                                        axis=mybir.AxisListType.X, op=OP.add)
                delta = cst.tile([128, 1], F32, name=f"{name}_dl{m}")
                nc.vector.tensor_copy(delta[:], zraw[m][:, nloc - 1:nloc])
                t1 = cst.tile([128, 1], F32, name=f"{name}_t1{m}")
                nc.vector.tensor_tensor(out=t1[:], in0=delta[:], in1=n_emp,
                                        op=OP.mult)
                nc.vector.tensor_tensor(out=arin[:, m:m + 1], in0=rs_[:, 0:1],
                                        in1=t1[:], op=OP.subtract)
                d2 = cst.tile([128, 1], F32, name=f"{name}_d2{m}")
                nc.vector.tensor_tensor(out=d2[:], in0=delta[:], in1=t1[:],
                                        op=OP.mult)
                nc.vector.tensor_tensor(out=arin[:, mparts + m:mparts + m + 1],
                                        in0=rs_[:, 1:2], in1=d2[:],
                                        op=OP.subtract)
            st = allreduce(arin[:], 2 * mparts, f"{name}_ar")
            zout = []
            scsh = []
            for m in range(mparts):
                scsh.append(bn_params(st[:, m:m + 1],
                                      st[:, mparts + m:mparts + m + 1],
                                      g_aps[m], be_aps[m], f"{name}_p{m}"))
                zout.append(big.tile([128, nloc], BF16, name=f"{name}_n{m}",
                                     tag=zout_tags[m][1]))
            for o, w in blocks:
                for m in range(mparts):
                    sc, sh = scsh[m]
                    if m == 0:
                        nc.scalar.activation(zout[m][:, o:o + w],
                                             zraw[m][:, o:o + w], AF.Relu,
                                             bias=sh[:, 0:1], scale=sc[:, 0:1])
                    else:
                        # relu(zraw*sc+sh) as two fast tensor_scalar passes
                        tmp = wk.tile([128, 512], BF16, name=f"{name}_rt",
                                      tag="sqscr", bufs=2)
                        nc.vector.tensor_scalar(out=tmp[:, :w],
                                                in0=zraw[m][:, o:o + w],
                                                scalar1=sc[:, 0:1],
                                                scalar2=sh[:, 0:1],
                                                op0=OP.mult, op1=OP.add)
                        nc.vector.tensor_scalar(out=zout[m][:, o:o + w],
                                                in0=tmp[:, :w], scalar1=0.0,
                                                scalar2=None, op0=OP.max)
            return zout

        # L1: z0 [C,n] -> z1 halves; W1 [128, 2H]: lhsT for half m = w1[:, m*128...]
        z1 = mlp_layer([z0], w1, lambda k, m: m * 128, 1,
                       [("slotA", "slotB"), ("slotC", "slotD")],
                       b1, g1, be1, "l1")
        # L2: z1 (2 K-parts) -> z2 halves; W2sb [128, 512]:
        # lhsT(k,m) at col k*256 + m*128
        z2 = mlp_layer(z1, w2, lambda k, m: k * 256 + m * 128, 2,
                       [("slotA", "slotB"), ("slotC", "slotD")],
                       b2, g2, be2, "l2")

        # ---- phase 7: z3 = W3^T @ z2 + b3; u = z3 + xT; out = We^T @ u + be
        for o, w in blocks:
            ps3 = ps.tile([128, 512], F32, name="ps3", tag="mm", bufs=3)
            for k in range(2):
                nc.tensor.matmul(
                    ps3[:, :w], lhsT=w3[:, k * 128:k * 128 + 128],
                    rhs=z2[k][:, o:o + w],
                    start=(k == 0), stop=(k == 1))
            # u2 = (ps3 + b3) + xT fused on DVE (frees ACT for L2 relus)
            u2 = wk.tile([128, 512], BF16, name="u2", tag="u2", bufs=3)
            nc.vector.scalar_tensor_tensor(
                out=u2[:, :w], in0=ps3[:, :w], scalar=b3,
                in1=xT[:, o:o + w], op0=OP.add, op1=OP.add)
            ps4 = ps.tile([128, 512], F32, name="ps4", tag="mm2", bufs=2)
            nc.tensor.matmul(ps4[:, :w], lhsT=we[:], rhs=u2[:, :w],
                             start=True, stop=True)
            ob = wk.tile([128, 512], F32, name="ob", tag="ob", bufs=2)
            nc.vector.tensor_scalar(out=ob[:, :w], in0=ps4[:, :w],
                                    scalar1=be_enc, scalar2=0.0,
                                    op0=OP.add, op1=OP.add)
            nc.sync.dma_start(out_d[:, o:o + w], ob[:, :w])

    nc.compile()
    return nc


# --------------------------------------------------------------------------
# public entry
# --------------------------------------------------------------------------

_CACHE = {}
LAST_RESULT = None


def _run(x, edge_index, bn_g, bn_b, W1, b1, g1, be1, W2, b2, g2, be2,
         W3, b3, We, be, n_nodes):
    import ml_dtypes
    bf16 = ml_dtypes.bfloat16

    src = np.asarray(edge_index[0], dtype=np.int64)
    dst = np.asarray(edge_index[1], dtype=np.int64)
    x = np.asarray(x, dtype=np.float32)

    plan = _plan(n_nodes, src, dst)
    tiles, nloc, NB = plan["tiles"], plan["nloc"], plan["NB"]

    key = (tiles, NB, tuple(plan["B"].tolist()), n_nodes)
    if key not in _CACHE:
        _CACHE[key] = _build(plan, n_nodes)
    nc = _CACHE[key]

    h_dim = W1.shape[1]
    # pack weights
    w2sb = np.ascontiguousarray(
        W2.reshape(2, 128, h_dim).transpose(1, 0, 2).reshape(128, 2 * h_dim)
    ).astype(bf16)
    w3sb = np.ascontiguousarray(
        W3.reshape(2, 128, 128).transpose(1, 0, 2).reshape(128, 256)
    ).astype(bf16)

    halves = lambda v: [np.asarray(v[:128], np.float32),
                        np.asarray(v[128:], np.float32)]
    b1h, g1h, be1h = halves(b1), halves(g1), halves(be1)
    b2h, g2h, be2h = halves(b2), halves(g2), halves(be2)

    nslots = NB * 128

    in_maps = []
    core_nodes = []
    for c in range(NCORES):
        sel = plan["core_of"] == c
        nodes_c = np.nonzero(sel)[0]
        core_nodes.append(nodes_c)
        xr = np.zeros((nloc, 128), np.float32)
        xr[plan["loc_of"][nodes_c]] = x[nodes_c]

        # host pre-gather (pure indexing): edge-major x[src] tiles, bf16.
        # edge slot e = b*128 + lane -> SBUF [lane, b, ch]
        esrc = plan["esrc"][c]
        xe = x[esrc].astype(bf16)                            # [nslots, 128]
        xg = np.ascontiguousarray(
            xe.reshape(NB, 128, 128).transpose(1, 0, 2).reshape(128, nslots))

        # one-hot scatter matrix S: [lane, block, dst_slot]
        ed = plan["edslot"][c]
        S2 = np.zeros((nslots, 128), np.float32)
        valid = np.nonzero(ed >= 0)[0]
        S2[valid, ed[valid]] = 1.0
        Sm = np.ascontiguousarray(
            S2.reshape(NB, 128, 128).transpose(1, 0, 2).reshape(128, nslots)
        ).astype(ml_dtypes.float8_e4m3)

        pcols = np.zeros((128, 17), np.float32)
        pcols[:, 0] = bn_g
        pcols[:, 1] = bn_b
        for i2 in range(2):
            pcols[:, 2 + i2] = b1h[i2]
            pcols[:, 4 + i2] = g1h[i2]
            pcols[:, 6 + i2] = be1h[i2]
            pcols[:, 8 + i2] = b2h[i2]
            pcols[:, 10 + i2] = g2h[i2]
            pcols[:, 12 + i2] = be2h[i2]
        pcols[:, 14] = b3
        pcols[:, 15] = be
        pcols[:, 16] = plan["n_empty"][c]
        in_maps.append({
            "xT": np.ascontiguousarray(xr.T),
            "xg": xg,
            "sm": Sm,
            "w1": np.asarray(W1, np.float32).astype(bf16),
            "w2": w2sb,
            "w3": w3sb,
            "we": np.asarray(We, np.float32).astype(bf16),
            "pcols": pcols,
        })

    import os
    trace = bool(os.environ.get("KTRACE"))
    res = run_bass_kernel_spmd(nc, in_maps, list(range(NCORES)), trace=trace)
    global LAST_RESULT
    LAST_RESULT = res
    out = np.empty((n_nodes, 128), np.float32)
    for c in range(NCORES):
        nodes_c = core_nodes[c]
        out[nodes_c] = res.results[c]["outT"][:, plan["loc_of"][nodes_c]].T
    return out


def kernel(x, edge_index, bn_g, bn_b, W1, b1, g1, be1, W2, b2, g2, be2,
           W3, b3, We, be):
    return _run(x, edge_index, bn_g, bn_b, W1, b1, g1, be1, W2, b2, g2, be2,
                W3, b3, We, be, n_nodes=x.shape[0])
